# revision 1
# baseline (speedup 1.0000x reference)
"""Trainium2 Bass kernel for nn_ConvTM2d (Tsetlin-machine conv layer).

Reference computation (jax, fp32):
    patches = unfold(x, 3x3, pad=1)                        # [B, 576, 3136]
    lits    = [patches, 1-patches]                         # [B*L, 1152]
    include = (sigmoid(w_include) > 0.5)                   # binary [256, 1152]
    clauses = exp(log(lits + 1e-6) @ include.T)            # [B*L, 256]
    logits  = clauses @ vote.T                             # [B*L, 128]
    out     = logits -> [B, 128, 56, 56]

Device formulation: the unfold+GEMM is a 3x3 convolution over a 128-channel
"log literal" image (64 ch of log(x+eps), 64 ch of log(1-x+eps)), evaluated
as 9 shifted matmuls accumulating in PSUM, followed by exp (ACT) and a 1x1
conv with vote (2 more matmuls). Zero-padding in x-space becomes constant
borders in log-space (log(eps) / log(1+eps)).

Sharding: data-parallel over batch B=16 -> 2 batches per core x 8 cores.
w_include binarization + transpose into matmul-stationary layout is done
once on host (tiny: ~300K elements).

Schedule notes:
 - input x is DMA'd in 8-row slabs; complement literals come from a
   cross-partition DVE op (write p64-127 from p0-63); Ln runs per slab so
   the first conv matmuls start as soon as rows 0-9 of the padded
   log-literal image exist.
 - all Ln slabs (both batches) are emitted before any Exp so the ACT
   engine loads each activation table exactly once.
 - a burst of dummy matmuls at t~0 keeps the PE busy through the HAM
   activity window so real matmuls run at 2.4 GHz from the start.
"""

import numpy as np
import ml_dtypes

EPS = 1e-6
B_FULL = 16
N_CORES = 8
B_PER_CORE = B_FULL // N_CORES
C_IN = 64
H = W = 56
HP = WP = 58  # padded
N_CLAUSES = 256
N_CLASSES = 128
R_TILE = 8  # output rows per matmul tile -> free dim 8*56=448
N_RTILES = H // R_TILE
N_WARM_MM = 13

LOG_EPS = float(np.log(EPS))            # border, x-literal half
LOG_1PEPS = float(np.log(1.0 + EPS))    # border, complement half

_CACHE = {}

# Test-only: scale log-literals by this factor so clause sums don't underflow
# exp() and the full dataflow can be validated numerically. None in production.
_DEBUG_SCALE = None


def _build_program():
    import concourse.bacc as bacc
    import concourse.mybir as mybir
    import concourse.tile as tile
    from concourse._compat import get_trn_type

    f32 = mybir.dt.float32
    bf16 = mybir.dt.bfloat16
    Ln = mybir.ActivationFunctionType.Ln
    Exp = mybir.ActivationFunctionType.Exp
    mult = mybir.AluOpType.mult
    add = mybir.AluOpType.add

    nc = bacc.Bacc(
        get_trn_type() or "TRN2",
        target_bir_lowering=False,
        debug=False,
        enable_asserts=False,
    )

    x_d = nc.dram_tensor("xs", [B_PER_CORE, C_IN, H, W], f32, kind="ExternalInput")
    w_d = nc.dram_tensor("wstat", [128, 9, N_CLAUSES], bf16, kind="ExternalInput")
    v_d = nc.dram_tensor("voteT", [128, N_CLAUSES], bf16, kind="ExternalInput")
    y_d = nc.dram_tensor("y", [B_PER_CORE, N_CLASSES, H, W], f32, kind="ExternalOutput")

    with tile.TileContext(nc) as tc:
        with tc.tile_pool(name="wpool", bufs=1) as wpool, \
             tc.tile_pool(name="xpool", bufs=2) as xpool, \
             tc.tile_pool(name="lpool", bufs=2) as lpool, \
             tc.tile_pool(name="cpool", bufs=6) as cpool, \
             tc.tile_pool(name="opool", bufs=4) as opool:

            # --- constants / dummy tiles (DVE is idle early) ---
            wzb = wpool.tile([128, R_TILE * W], bf16)
            nc.vector.memset(wzb[:], 0.0)
            eps_t = wpool.tile([128, 1], f32)
            nc.vector.memset(eps_t[:], EPS)

            # Pre-load the Ln ACT table off the critical path with a dummy
            # activation that only depends on the eps memset.
            dact = wpool.tile([128, 1], f32)
            ln_insts, exp_insts = [], []
            ln_insts.append(
                nc.scalar.activation(dact[:], eps_t[:], Ln, bias=eps_t[:]))

            # --- PE HAM warmup: ~3.5us of small dummy matmuls starting right
            # after the engine preamble so the PE is at 2.4GHz when the first
            # real matmul issues. The pool closes afterwards, freeing its
            # PSUM bank for cpsum.
            with tc.tile_pool(name="warmps", bufs=1, space="PSUM") as warmps:
                wps = warmps.tile([64, R_TILE * W], f32)
                for _ in range(N_WARM_MM):
                    nc.tensor.matmul(wps[:], wzb[:, 0:64], wzb[:],
                                     start=True, stop=True)

            # --- weights in (sync queue; x goes on the gpsimd queue) ---
            # wstat lands in two pieces so the first LDWEIGHTS isn't gated on
            # the whole 590KB.
            wsb = wpool.tile([128, 9, N_CLAUSES], bf16)
            nc.sync.dma_start(wsb[:, 0:3, :], w_d[:, 0:3, :])
            nc.sync.dma_start(wsb[:, 3:9, :], w_d[:, 3:9, :])
            vsb = wpool.tile([128, N_CLAUSES], bf16)
            nc.sync.dma_start(vsb[:], v_d[:])

            # --- log-literal images ---
            # x arrives in 2 chunks per batch (fewer, bigger DMAs: the ~600ns
            # per-DMA issue cost on an engine queue adds up). Batch 1's DMAs
            # are held back behind batch 0's second Ln slab so they don't
            # steal HBM bandwidth from the critical-path chunk.
            # x chunk schedule: (batch, lo, hi, index of Ln gating the DMA or
            # None for immediate issue). Later chunks are held behind early Ln
            # slabs so the critical rows-0-15 + wstat transfers get the full
            # HBM bandwidth; each chunk still lands before the slab that
            # needs it.
            x2s, Ls = [], []
            held = []  # (dma_inst, gate_ln_index)
            for b in range(B_PER_CORE):
                x2 = xpool.tile([128, H, W], f32, tag="x2")
                L = lpool.tile([128, HP, WP], bf16, tag="L")
                x2s.append(x2)
                Ls.append(L)
            chunk_plan = [
                (0, 0, 16, None),   # slabs 0-1: critical path
                (0, 16, 56, None),  # rest of b0, queued behind on gpsimd
                (1, 0, 28, 1),      # b1 held until b0 slab-0 Ln ran
                (1, 28, 56, 2),
            ]
            dma_gates = []
            for b, lo, hi, gate in chunk_plan:
                # immediate chunk on the gpsimd queue; held chunks on sync so
                # their semaphore waits don't block the border memsets below
                eng = nc.gpsimd if gate is None else nc.sync
                dma = eng.dma_start(
                    x2s[b][0:64, lo:hi, :], x_d[b, :, lo:hi, :])
                if gate is not None:
                    dma_gates.append((dma, gate))
            for b in range(B_PER_CORE):
                L = Ls[b]
                # borders: log(0 + eps) on x-half, log(1 - 0 + eps) on 1-x half
                nc.gpsimd.memset(L[0:64, 0, :], LOG_EPS)
                nc.gpsimd.memset(L[64:128, 0, :], LOG_1PEPS)
                nc.gpsimd.memset(L[0:64, HP - 1, :], LOG_EPS)
                nc.gpsimd.memset(L[64:128, HP - 1, :], LOG_1PEPS)
                nc.gpsimd.memset(L[0:64, 1:HP - 1, 0], LOG_EPS)
                nc.gpsimd.memset(L[64:128, 1:HP - 1, 0], LOG_1PEPS)
                nc.gpsimd.memset(L[0:64, 1:HP - 1, WP - 1], LOG_EPS)
                nc.gpsimd.memset(L[64:128, 1:HP - 1, WP - 1], LOG_1PEPS)
            for b in range(B_PER_CORE):
                x2, L = x2s[b], Ls[b]
                for s in range(N_RTILES):
                    r0 = s * R_TILE
                    sl = slice(r0, r0 + R_TILE)
                    # complement literals: p64-127 <- 1 - p0-63
                    nc.vector.tensor_scalar(
                        x2[64:128, sl, :], x2[0:64, sl, :], -1.0, 1.0, mult, add)
                    ln_insts.append(nc.scalar.activation(
                        L[:, 1 + r0:1 + r0 + R_TILE, 1:WP - 1], x2[:, sl, :],
                        Ln, bias=eps_t[:]))
                if _DEBUG_SCALE is not None:
                    nc.vector.tensor_scalar_mul(L[:], L[:], float(_DEBUG_SCALE))
            for dma, gate in dma_gates:
                tile.add_dep_helper(dma.ins, ln_insts[gate].ins, sync=True,
                                    reason="stagger x DMA behind critical path")

            # --- conv + exp + vote ---
            with tc.tile_pool(name="cpsum", bufs=7, space="PSUM") as cpsum, \
                 tc.tile_pool(name="lpsum", bufs=1, space="PSUM") as lpsum:
                for b in range(B_PER_CORE):
                    L = Ls[b]
                    for r in range(N_RTILES):
                        r0 = r * R_TILE
                        cls = []
                        for cc in range(2):
                            cps = cpsum.tile([128, R_TILE, W], f32)
                            for ij in range(9):
                                i, j = divmod(ij, 3)
                                nc.tensor.matmul(
                                    cps[:],
                                    wsb[:, ij, cc * 128:(cc + 1) * 128],
                                    L[:, r0 + i:r0 + i + R_TILE, j:j + W],
                                    start=(ij == 0),
                                    stop=(ij == 8),
                                )
                            C = cpool.tile([128, R_TILE, W], bf16)
                            exp_insts.append(
                                nc.scalar.activation(C[:], cps[:], Exp))
                            cls.append(C)
                        lps = lpsum.tile([128, R_TILE, W], f32)
                        nc.tensor.matmul(lps[:], vsb[:, 0:128], cls[0][:],
                                         start=True, stop=False)
                        nc.tensor.matmul(lps[:], vsb[:, 128:256], cls[1][:],
                                         start=False, stop=True)
                        o = opool.tile([128, R_TILE, W], f32)
                        nc.vector.tensor_copy(o[:], lps[:])
                        nc.sync.dma_start(y_d[b, :, r0:r0 + R_TILE, :], o[:])

                # Keep ACT phases contiguous (all Ln, then all Exp): a stray
                # Ln between Exps costs two 1.3us ACT_TABLE_LOADs + PE stall.
                for e in exp_insts:
                    tile.add_dep_helper(e.ins, ln_insts[-1].ins, sync=False,
                                        reason="ACT table phase order")

    nc.compile()
    return nc


def _host_prep(w_include, vote):
    bf16 = ml_dtypes.bfloat16
    include = (w_include > 0.0).astype(np.float32)  # sigmoid(w) > 0.5 <=> w > 0
    incT = np.ascontiguousarray(include.T)          # [1152, 256]
    top = incT[:576].reshape(C_IN, 9, N_CLAUSES)    # x-literal half, [c, ij, m]
    bot = incT[576:].reshape(C_IN, 9, N_CLAUSES)    # complement half
    wstat = np.ascontiguousarray(
        np.concatenate([top, bot], axis=0)).astype(bf16)  # [128, 9, 256]

    voteT = np.ascontiguousarray(vote.T)            # [256, 128] = [clause, class]
    vT = np.ascontiguousarray(
        np.concatenate([voteT[0:128], voteT[128:256]], axis=1)).astype(bf16)
    return wstat, vT


def kernel(x, w_include, vote, _trace=False):
    from concourse import bass_utils

    x = np.ascontiguousarray(np.asarray(x, dtype=np.float32))
    wstat, vT = _host_prep(np.asarray(w_include, dtype=np.float32),
                           np.asarray(vote, dtype=np.float32))

    if "nc" not in _CACHE:
        _CACHE["nc"] = _build_program()
    nc = _CACHE["nc"]

    in_maps = [
        {
            "xs": np.ascontiguousarray(
                x[core * B_PER_CORE:(core + 1) * B_PER_CORE]),
            "wstat": wstat,
            "voteT": vT,
        }
        for core in range(N_CORES)
    ]
    res = bass_utils.run_bass_kernel_spmd(
        nc, in_maps, core_ids=list(range(N_CORES)), trace=_trace,
    )
    out = np.concatenate([r["y"] for r in res.results], axis=0)
    if _trace:
        _CACHE["last_results"] = res
    return out



# revision 5
# speedup vs baseline: 1.2257x; 1.2257x over previous
"""Trainium2 Bass kernel for nn_ConvTM2d (Tsetlin-machine conv layer).

Reference computation (jax, fp32):
    patches = unfold(x, 3x3, pad=1)                        # [B, 576, 3136]
    lits    = [patches, 1-patches]                         # [B*L, 1152]
    include = (sigmoid(w_include) > 0.5)                   # binary [256, 1152]
    clauses = exp(log(lits + 1e-6) @ include.T)            # [B*L, 256]
    logits  = clauses @ vote.T                             # [B*L, 128]
    out     = logits -> [B, 128, 56, 56]

Device formulation (v2, fp8):
  - log-literal image L in fp8e4 [128ch, 2batch, 58, 64]: 64 ch of
    log(x+eps) + 64 ch of log(1-x+eps); row stride padded to 64 so
    vertically adjacent taps sit 64 B apart (DoubleRow pair stride must be
    16B-aligned). Zero-padding in x-space becomes constant borders in
    log-space.
  - conv as 6 matmuls per (rtile, clause-half): 3 fp8 DoubleRow matmuls
    covering tap rows 0+1 (pair dim = 2 K-subtiles -> K=256/instr) and 3
    plain fp8 matmuls for tap row 2. All accumulate into one PSUM tile
    [128, 1024] spanning 2 banks (half cc0 at 0, cc1 at 512).
  - one Exp per rtile over both halves at once [128, 2, 448] -> C fp8.
  - vote as ONE DoubleRow matmul per rtile (pair dim = the two clause
    halves), output written into the just-freed first PSUM bank of the
    same tile; DVE copies logits to bf16 and the host upcasts to fp32.
  - batch dim lives in the free dims (2 images per core), so Ln /
    complement / x DMA each cover both batches per instruction.

Sharding: data-parallel over batch B=16 -> 2 batches per core x 8 cores.
Weight binarization + fp8 packing is done once on host (tiny: ~300K elems).
"""

import numpy as np
import ml_dtypes

EPS = 1e-6
B_FULL = 16
N_CORES = 8
B_PER_CORE = B_FULL // N_CORES
C_IN = 64
H = W = 56
HP = 58          # padded image rows/cols (1px border)
HL, WL = 58, 64  # L image: rows x row-stride
N_CLAUSES = 256
N_CLASSES = 128
R_TILE = 8
N_RTILES = H // R_TILE           # 7
N_UNITS = B_PER_CORE * N_RTILES  # 14
FD = R_TILE * W                  # 448 free elems per unit
N_WARM_MM = 13
PS_LAG = 2   # vote for unit u emitted after conv of unit u+PS_LAG

LOG_EPS = float(np.log(EPS))
LOG_1PEPS = float(np.log(1.0 + EPS))

_CACHE = {}

# Test-only: scale log-literals by this factor so clause sums don't underflow
# exp() and the full dataflow can be validated numerically. None in production.
_DEBUG_SCALE = None

# conv matmul plan per clause half: (kind, i0, j)
#   "dr":  DoubleRow pair taps (i0,j)+(i0+1,j), pair stride WL
#   "pl":  plain fp8 matmul, tap (i0,j)
MM_PLAN = [
    ("dr", 0, 0), ("dr", 0, 1), ("dr", 0, 2),
    ("pl", 2, 0), ("pl", 2, 1), ("pl", 2, 2),
]


def _build_program():
    import concourse.bacc as bacc
    import concourse.bass as bass
    import concourse.mybir as mybir
    import concourse.tile as tile
    from concourse._compat import get_trn_type

    f32 = mybir.dt.float32
    bf16 = mybir.dt.bfloat16
    fp8 = mybir.dt.float8e4
    Ln = mybir.ActivationFunctionType.Ln
    Exp = mybir.ActivationFunctionType.Exp
    DR = mybir.MatmulPerfMode.DoubleRow
    mult = mybir.AluOpType.mult
    add = mybir.AluOpType.add

    nc = bacc.Bacc(
        get_trn_type() or "TRN2",
        target_bir_lowering=False,
        debug=False,
        enable_asserts=False,
    )

    x_d = nc.dram_tensor("xs", [C_IN, B_PER_CORE, H, W], bf16,
                         kind="ExternalInput")
    w_d = nc.dram_tensor("wstat", [128, 12, 2, 128], fp8,
                         kind="ExternalInput")
    v_d = nc.dram_tensor("voteT", [128, 2, 128], fp8, kind="ExternalInput")
    y_d = nc.dram_tensor("y", [B_PER_CORE, N_CLASSES, H, W], bf16,
                         kind="ExternalOutput")

    def conv_rhs(L, b, r0, i0, j, pair):
        """moving-operand AP into L [128, 2, HL, WL] fp8 for tap (i0,j) of
        rtile r0, batch b; pair adds the DoubleRow dim (taps i0, i0+1)."""
        base = L[:, 0, 0, 0]
        off = L.offset + b * (HL * WL) + (r0 + i0) * WL + j
        dims = [list(L.ap[0])]
        if pair:
            dims.append([WL, 2])
        dims += [[WL, R_TILE], [1, W]]
        return bass.AP(base.tensor, off, dims)

    with tile.TileContext(nc) as tc:
        with tc.tile_pool(name="wpool", bufs=1) as wpool, \
             tc.tile_pool(name="xpool", bufs=1) as xpool, \
             tc.tile_pool(name="lpool", bufs=1) as lpool, \
             tc.tile_pool(name="cpool", bufs=3) as cpool, \
             tc.tile_pool(name="opool", bufs=3) as opool:

            # --- constants / warmup ---
            wzb = wpool.tile([128, FD], bf16)
            nc.vector.memset(wzb[:], 0.0)
            eps_t = wpool.tile([128, 1], f32)
            nc.vector.memset(eps_t[:], EPS)

            # Pre-load the Ln ACT table off the critical path.
            dact = wpool.tile([128, 1], f32)
            ln_insts, exp_insts = [], []
            ln_insts.append(
                nc.scalar.activation(dact[:], eps_t[:], Ln, bias=eps_t[:]))

            # PE HAM warmup: ~4us of dummy matmuls so the PE runs at 2.4GHz
            # when the first real matmul issues. Pool closes afterwards,
            # freeing its PSUM bank for cpsum.
            with tc.tile_pool(name="warmps", bufs=1, space="PSUM") as warmps:
                wps = warmps.tile([64, FD], f32)
                for _ in range(N_WARM_MM):
                    nc.tensor.matmul(wps[:], wzb[:, 0:64], wzb[:],
                                     start=True, stop=True)

            # --- weights + x in ---
            wsb = wpool.tile([128, 12, 2, 128], fp8)
            vsb = wpool.tile([128, 2, 128], fp8)
            nc.sync.dma_start(wsb[:], w_d[:])
            nc.sync.dma_start(vsb[:], v_d[:])

            xsb = xpool.tile([128, B_PER_CORE, H, W], bf16, name="xsb")
            Lt = lpool.tile([128, B_PER_CORE, HL, WL], fp8, name="Lt")
            if _DEBUG_SCALE is not None:
                # debug-only: the scale pass reads the whole tile incl. the
                # unused stride-padding columns
                nc.gpsimd.memset(Lt[:], 0.0)

            # x in 4 row-chunks; first two on the sync queue (critical
            # path), rest on the gpsimd queue.
            chunks = [(0, 8, nc.sync), (8, 24, nc.sync),
                      (24, 40, nc.gpsimd), (40, 56, nc.gpsimd)]
            for lo, hi, eng in chunks:
                eng.dma_start(xsb[0:64, :, lo:hi, :], x_d[:, :, lo:hi, :])

            # --- L border memsets (fp8 constants) ---
            for half, val in ((slice(0, 64), LOG_EPS),
                              (slice(64, 128), LOG_1PEPS)):
                nc.gpsimd.memset(Lt[half, :, 0, :], val)
                nc.gpsimd.memset(Lt[half, :, HP - 1, :], val)
                nc.vector.memset(Lt[half, :, 1:HP - 1, 0], val)
                nc.vector.memset(Lt[half, :, 1:HP - 1, HP - 1], val)

            # --- log-literal production (8-row slabs, both batches) ---
            for s in range(N_RTILES):
                r0 = s * R_TILE
                sl = slice(r0, r0 + R_TILE)
                nc.vector.tensor_scalar(
                    xsb[64:128, :, sl, :], xsb[0:64, :, sl, :],
                    -1.0, 1.0, mult, add)
                ln_insts.append(nc.scalar.activation(
                    Lt[:, :, 1 + r0:1 + r0 + R_TILE, 1:1 + W],
                    xsb[:, :, sl, :], Ln, bias=eps_t[:]))
            if _DEBUG_SCALE is not None:
                nc.vector.tensor_scalar_mul(Lt[:], Lt[:],
                                            float(_DEBUG_SCALE))

            # --- conv + exp + vote + out, software-pipelined per unit ---
            with tc.tile_pool(name="cpsum", bufs=4, space="PSUM") as cpsum:
                units = [(b, r) for b in range(B_PER_CORE)
                         for r in range(N_RTILES)]
                cps_tiles = [None] * N_UNITS
                C_tiles = [None] * N_UNITS

                def emit_conv(u):
                    b, r = units[u]
                    r0 = r * R_TILE
                    cps = cpsum.tile([128, 1024], f32, name="cps")
                    cps_tiles[u] = cps
                    for cc in range(2):
                        for mi, (kind, i0, j) in enumerate(MM_PLAN):
                            widx = cc * 6 + mi
                            pair = kind == "dr"
                            rhs = conv_rhs(Lt, b, r0, i0, j, pair)
                            lhsT = (wsb[:, widx, :, :] if pair
                                    else wsb[:, widx, 0, :])
                            nc.tensor.matmul(
                                cps[:, cc * 512:cc * 512 + FD],
                                lhsT, rhs,
                                start=(mi == 0),
                                stop=(mi == len(MM_PLAN) - 1),
                                perf_mode=(DR if pair else None),
                            )
                    # exp over both halves in one ACT op -> fp8 C
                    C = cpool.tile([128, 2, FD], fp8, name="C")
                    C_tiles[u] = C
                    src = bass.AP(cps.tensor, cps.offset,
                                  [list(cps.ap[0]), [512, 2], [1, FD]])
                    exp_insts.append(nc.scalar.activation(C[:], src, Exp))

                def emit_vote(u):
                    b, r = units[u]
                    r0 = r * R_TILE
                    cps = cps_tiles[u]
                    nc.tensor.matmul(
                        cps[:, 0:FD], vsb[:, :, :], C_tiles[u][:, :, :],
                        start=True, stop=True, perf_mode=DR,
                    )
                    o = opool.tile([128, FD], bf16, name="o")
                    nc.vector.tensor_copy(o[:], cps[:, 0:FD])
                    nc.gpsimd.dma_start(y_d[b, :, r0:r0 + R_TILE, :], o[:])

                for u in range(N_UNITS):
                    emit_conv(u)
                    if u >= PS_LAG:
                        emit_vote(u - PS_LAG)
                for u in range(N_UNITS - PS_LAG, N_UNITS):
                    emit_vote(u)

                # ACT phases contiguous: all Ln, then all Exp (one table
                # switch instead of thrashing).
                for e in exp_insts:
                    tile.add_dep_helper(e.ins, ln_insts[-1].ins, sync=False,
                                        reason="ACT table phase order")

    nc.compile()
    return nc


def _lit_index(k, i, j):
    """w_include column for literal (channel-partition k, tap (i,j))."""
    if k < 64:
        return k * 9 + i * 3 + j
    return 576 + (k - 64) * 9 + i * 3 + j


def _host_prep(w_include, vote):
    fp8 = ml_dtypes.float8_e4m3
    include = (w_include > 0.0).astype(np.float32)  # sigmoid(w)>0.5 <=> w>0

    # wstat [128, 12, 2, 128]: widx = cc*6 + mi over MM_PLAN
    wstat = np.zeros((128, 12, 2, 128), np.float32)
    ks = np.arange(128)
    for cc in range(2):
        for mi, (kind, i0, j) in enumerate(MM_PLAN):
            widx = cc * 6 + mi
            taps = [(i0, j), (i0 + 1, j)] if kind == "dr" else [(i0, j)]
            for p, (i, jj) in enumerate(taps):
                cols = np.array([_lit_index(k, i, jj) for k in ks])
                # [k, m] = include[cc*128+m, cols[k]]
                wstat[:, widx, p, :] = include[cc * 128:(cc + 1) * 128,
                                               cols].T

    # voteT [128, 2, 128]: [k, half, class] = vote[class, half*128 + k]
    voteT = np.empty((128, 2, 128), np.float32)
    for i in range(2):
        voteT[:, i, :] = vote[:, i * 128:(i + 1) * 128].T
    np.clip(voteT, -240.0, 240.0, out=voteT)

    return wstat.astype(fp8), voteT.astype(fp8)


def kernel(x, w_include, vote, _trace=False):
    from concourse import bass_utils

    x = np.asarray(x, dtype=np.float32)
    wstat, vT = _host_prep(np.asarray(w_include, dtype=np.float32),
                           np.asarray(vote, dtype=np.float32))

    if "nc" not in _CACHE:
        _CACHE["nc"] = _build_program()
    nc = _CACHE["nc"]

    in_maps = []
    for core in range(N_CORES):
        xs = x[core * B_PER_CORE:(core + 1) * B_PER_CORE]
        xs = np.ascontiguousarray(
            xs.transpose(1, 0, 2, 3)).astype(ml_dtypes.bfloat16)
        in_maps.append({"xs": xs, "wstat": wstat, "voteT": vT})

    res = bass_utils.run_bass_kernel_spmd(
        nc, in_maps, core_ids=list(range(N_CORES)), trace=_trace,
    )
    out = np.concatenate(
        [r["y"].astype(np.float32) for r in res.results], axis=0)
    if _trace:
        _CACHE["last_results"] = res
    return out


# revision 6
# speedup vs baseline: 1.2745x; 1.0398x over previous
"""Trainium2 Bass kernel for nn_ConvTM2d (Tsetlin-machine conv layer).

Reference computation (jax, fp32):
    patches = unfold(x, 3x3, pad=1)                        # [B, 576, 3136]
    lits    = [patches, 1-patches]                         # [B*L, 1152]
    include = (sigmoid(w_include) > 0.5)                   # binary [256, 1152]
    clauses = exp(log(lits + 1e-6) @ include.T)            # [B*L, 256]
    logits  = clauses @ vote.T                             # [B*L, 128]
    out     = logits -> [B, 128, 56, 56]

Device formulation (v2, fp8):
  - log-literal image L in fp8e4 [128ch, 2batch, 58, 64]: 64 ch of
    log(x+eps) + 64 ch of log(1-x+eps); row stride padded to 64 so
    vertically adjacent taps sit 64 B apart (DoubleRow pair stride must be
    16B-aligned). Zero-padding in x-space becomes constant borders in
    log-space.
  - conv as 6 matmuls per (rtile, clause-half): 3 fp8 DoubleRow matmuls
    covering tap rows 0+1 (pair dim = 2 K-subtiles -> K=256/instr) and 3
    plain fp8 matmuls for tap row 2. All accumulate into one PSUM tile
    [128, 1024] spanning 2 banks (half cc0 at 0, cc1 at 512).
  - one Exp per rtile over both halves at once [128, 2, 448] -> C fp8.
  - vote as ONE DoubleRow matmul per rtile (pair dim = the two clause
    halves), output written into the just-freed first PSUM bank of the
    same tile; DVE copies logits to bf16 and the host upcasts to fp32.
  - batch dim lives in the free dims (2 images per core), so Ln /
    complement / x DMA each cover both batches per instruction.

Sharding: data-parallel over batch B=16 -> 2 batches per core x 8 cores.
Weight binarization + fp8 packing is done once on host (tiny: ~300K elems).
"""

import numpy as np
import ml_dtypes

EPS = 1e-6
B_FULL = 16
N_CORES = 8
B_PER_CORE = B_FULL // N_CORES
C_IN = 64
H = W = 56
HP = 58          # padded image rows/cols (1px border)
HL, WL = 58, 64  # L image: rows x row-stride
N_CLAUSES = 256
N_CLASSES = 128
R_TILE = 8
N_RTILES = H // R_TILE           # 7
N_UNITS = B_PER_CORE * N_RTILES  # 14
FD = R_TILE * W                  # 448 free elems per unit
N_WARM_MM = 10
PS_LAG = 2   # vote for unit u emitted after conv of unit u+PS_LAG

LOG_EPS = float(np.log(EPS))
LOG_1PEPS = float(np.log(1.0 + EPS))

_CACHE = {}

# Test-only: scale log-literals by this factor so clause sums don't underflow
# exp() and the full dataflow can be validated numerically. None in production.
_DEBUG_SCALE = None

# conv matmul plan per clause half: (kind, i0, j)
#   "dr":  DoubleRow pair taps (i0,j)+(i0+1,j), pair stride WL
#   "pl":  plain fp8 matmul, tap (i0,j)
MM_PLAN = [
    ("dr", 0, 0), ("dr", 0, 1), ("dr", 0, 2),
    ("pl", 2, 0), ("pl", 2, 1), ("pl", 2, 2),
]


def _build_program():
    import concourse.bacc as bacc
    import concourse.bass as bass
    import concourse.mybir as mybir
    import concourse.tile as tile
    from concourse._compat import get_trn_type

    f32 = mybir.dt.float32
    bf16 = mybir.dt.bfloat16
    fp8 = mybir.dt.float8e4
    Ln = mybir.ActivationFunctionType.Ln
    Exp = mybir.ActivationFunctionType.Exp
    DR = mybir.MatmulPerfMode.DoubleRow
    mult = mybir.AluOpType.mult
    add = mybir.AluOpType.add

    nc = bacc.Bacc(
        get_trn_type() or "TRN2",
        target_bir_lowering=False,
        debug=False,
        enable_asserts=False,
    )

    x_d = nc.dram_tensor("xs", [C_IN, B_PER_CORE, H, W], bf16,
                         kind="ExternalInput")
    w_d = nc.dram_tensor("wstat", [128, 12, 2, 128], fp8,
                         kind="ExternalInput")
    v_d = nc.dram_tensor("voteT", [128, 2, 128], fp8, kind="ExternalInput")
    y_d = nc.dram_tensor("y", [B_PER_CORE, N_CLASSES, H, W], bf16,
                         kind="ExternalOutput")

    def conv_rhs(L, b, r0, i0, j, pair):
        """moving-operand AP into L [128, 2, HL, WL] fp8 for tap (i0,j) of
        rtile r0, batch b; pair adds the DoubleRow dim (taps i0, i0+1)."""
        base = L[:, 0, 0, 0]
        off = L.offset + b * (HL * WL) + (r0 + i0) * WL + j
        dims = [list(L.ap[0])]
        if pair:
            dims.append([WL, 2])
        dims += [[WL, R_TILE], [1, W]]
        return bass.AP(base.tensor, off, dims)

    with tile.TileContext(nc) as tc:
        with tc.tile_pool(name="wpool", bufs=1) as wpool, \
             tc.tile_pool(name="xpool", bufs=1) as xpool, \
             tc.tile_pool(name="lpool", bufs=1) as lpool, \
             tc.tile_pool(name="cpool", bufs=3) as cpool, \
             tc.tile_pool(name="opool", bufs=3) as opool:

            # --- constants / warmup ---
            wzb = wpool.tile([128, FD], bf16)
            nc.vector.memset(wzb[:], 0.0)
            eps_t = wpool.tile([128, 1], f32)
            nc.vector.memset(eps_t[:], EPS)

            # Pre-load the Ln ACT table off the critical path.
            dact = wpool.tile([128, 1], f32)
            ln_insts, exp_insts = [], []
            ln_insts.append(
                nc.scalar.activation(dact[:], eps_t[:], Ln, bias=eps_t[:]))

            # PE HAM warmup: ~4us of dummy matmuls so the PE runs at 2.4GHz
            # when the first real matmul issues. Pool closes afterwards,
            # freeing its PSUM bank for cpsum.
            with tc.tile_pool(name="warmps", bufs=1, space="PSUM") as warmps:
                wps = warmps.tile([64, FD], f32)
                for _ in range(N_WARM_MM):
                    nc.tensor.matmul(wps[:], wzb[:, 0:64], wzb[:],
                                     start=True, stop=True)

            # --- weights + x in ---
            wsb = wpool.tile([128, 12, 2, 128], fp8)
            vsb = wpool.tile([128, 2, 128], fp8)
            nc.sync.dma_start(wsb[:], w_d[:])
            nc.sync.dma_start(vsb[:], v_d[:])

            Lt = lpool.tile([128, B_PER_CORE, HL, WL], fp8, name="Lt")
            if _DEBUG_SCALE is not None:
                # debug-only: the scale pass reads the whole tile incl. the
                # unused stride-padding columns
                nc.gpsimd.memset(Lt[:], 0.0)

            # x in 4 row-chunks, one SBUF tile per chunk so a chunk's
            # complement (write to partitions 64-127) never serializes
            # behind a later chunk's DMA (tile-granular WAW tracking).
            chunks = [(0, 16, nc.sync), (16, 32, nc.sync),
                      (32, 48, nc.gpsimd), (48, 56, nc.gpsimd)]
            xcks = []
            for ci, (lo, hi, eng) in enumerate(chunks):
                xc = xpool.tile([128, B_PER_CORE, 16, W], bf16,
                                name=f"xc{ci}", tag=f"xc{ci}")
                eng.dma_start(xc[0:64, :, 0:hi - lo, :], x_d[:, :, lo:hi, :])
                xcks.append(xc)

            # --- L border memsets (fp8 constants) ---
            for half, val in ((slice(0, 64), LOG_EPS),
                              (slice(64, 128), LOG_1PEPS)):
                nc.gpsimd.memset(Lt[half, :, 0, :], val)
                nc.gpsimd.memset(Lt[half, :, HP - 1, :], val)
                nc.vector.memset(Lt[half, :, 1:HP - 1, 0], val)
                nc.vector.memset(Lt[half, :, 1:HP - 1, HP - 1], val)

            # --- log-literal production (8-row slabs, both batches) ---
            for s in range(N_RTILES):
                r0 = s * R_TILE
                xc = xcks[r0 // 16]
                sl = slice(r0 % 16, r0 % 16 + R_TILE)
                nc.vector.tensor_scalar(
                    xc[64:128, :, sl, :], xc[0:64, :, sl, :],
                    -1.0, 1.0, mult, add)
                ln_insts.append(nc.scalar.activation(
                    Lt[:, :, 1 + r0:1 + r0 + R_TILE, 1:1 + W],
                    xc[:, :, sl, :], Ln, bias=eps_t[:]))
            if _DEBUG_SCALE is not None:
                nc.vector.tensor_scalar_mul(Lt[:], Lt[:],
                                            float(_DEBUG_SCALE))

            # --- conv + exp + vote + out, software-pipelined per unit ---
            with tc.tile_pool(name="cpsum", bufs=4, space="PSUM") as cpsum:
                units = [(b, r) for b in range(B_PER_CORE)
                         for r in range(N_RTILES)]
                cps_tiles = [None] * N_UNITS
                C_tiles = [None] * N_UNITS

                def emit_conv(u):
                    b, r = units[u]
                    r0 = r * R_TILE
                    cps = cpsum.tile([128, 1024], f32, name="cps")
                    cps_tiles[u] = cps
                    for cc in range(2):
                        for mi, (kind, i0, j) in enumerate(MM_PLAN):
                            widx = cc * 6 + mi
                            pair = kind == "dr"
                            rhs = conv_rhs(Lt, b, r0, i0, j, pair)
                            lhsT = (wsb[:, widx, :, :] if pair
                                    else wsb[:, widx, 0, :])
                            nc.tensor.matmul(
                                cps[:, cc * 512:cc * 512 + FD],
                                lhsT, rhs,
                                start=(mi == 0),
                                stop=(mi == len(MM_PLAN) - 1),
                                perf_mode=(DR if pair else None),
                            )
                    # exp over both halves in one ACT op -> fp8 C
                    C = cpool.tile([128, 2, FD], fp8, name="C")
                    C_tiles[u] = C
                    src = bass.AP(cps.tensor, cps.offset,
                                  [list(cps.ap[0]), [512, 2], [1, FD]])
                    exp_insts.append(nc.scalar.activation(C[:], src, Exp))

                def emit_vote(u):
                    b, r = units[u]
                    r0 = r * R_TILE
                    cps = cps_tiles[u]
                    nc.tensor.matmul(
                        cps[:, 0:FD], vsb[:, :, :], C_tiles[u][:, :, :],
                        start=True, stop=True, perf_mode=DR,
                    )
                    o = opool.tile([128, FD], bf16, name="o")
                    nc.vector.tensor_copy(o[:], cps[:, 0:FD])
                    nc.sync.dma_start(y_d[b, :, r0:r0 + R_TILE, :], o[:])

                for u in range(N_UNITS):
                    emit_conv(u)
                    if u >= PS_LAG:
                        emit_vote(u - PS_LAG)
                for u in range(N_UNITS - PS_LAG, N_UNITS):
                    emit_vote(u)

                # ACT phases contiguous: all Ln, then all Exp (one table
                # switch instead of thrashing).
                for e in exp_insts:
                    tile.add_dep_helper(e.ins, ln_insts[-1].ins, sync=False,
                                        reason="ACT table phase order")

    nc.compile()
    return nc


def _lit_index(k, i, j):
    """w_include column for literal (channel-partition k, tap (i,j))."""
    if k < 64:
        return k * 9 + i * 3 + j
    return 576 + (k - 64) * 9 + i * 3 + j


def _host_prep(w_include, vote):
    fp8 = ml_dtypes.float8_e4m3
    include = (w_include > 0.0).astype(np.float32)  # sigmoid(w)>0.5 <=> w>0

    # wstat [128, 12, 2, 128]: widx = cc*6 + mi over MM_PLAN
    wstat = np.zeros((128, 12, 2, 128), np.float32)
    ks = np.arange(128)
    for cc in range(2):
        for mi, (kind, i0, j) in enumerate(MM_PLAN):
            widx = cc * 6 + mi
            taps = [(i0, j), (i0 + 1, j)] if kind == "dr" else [(i0, j)]
            for p, (i, jj) in enumerate(taps):
                cols = np.array([_lit_index(k, i, jj) for k in ks])
                # [k, m] = include[cc*128+m, cols[k]]
                wstat[:, widx, p, :] = include[cc * 128:(cc + 1) * 128,
                                               cols].T

    # voteT [128, 2, 128]: [k, half, class] = vote[class, half*128 + k]
    voteT = np.empty((128, 2, 128), np.float32)
    for i in range(2):
        voteT[:, i, :] = vote[:, i * 128:(i + 1) * 128].T
    np.clip(voteT, -240.0, 240.0, out=voteT)

    return wstat.astype(fp8), voteT.astype(fp8)


def kernel(x, w_include, vote, _trace=False):
    from concourse import bass_utils

    x = np.asarray(x, dtype=np.float32)
    wstat, vT = _host_prep(np.asarray(w_include, dtype=np.float32),
                           np.asarray(vote, dtype=np.float32))

    if "nc" not in _CACHE:
        _CACHE["nc"] = _build_program()
    nc = _CACHE["nc"]

    in_maps = []
    for core in range(N_CORES):
        xs = x[core * B_PER_CORE:(core + 1) * B_PER_CORE]
        xs = np.ascontiguousarray(
            xs.transpose(1, 0, 2, 3)).astype(ml_dtypes.bfloat16)
        in_maps.append({"xs": xs, "wstat": wstat, "voteT": vT})

    res = bass_utils.run_bass_kernel_spmd(
        nc, in_maps, core_ids=list(range(N_CORES)), trace=_trace,
    )
    out = np.concatenate(
        [r["y"].astype(np.float32) for r in res.results], axis=0)
    if _trace:
        _CACHE["last_results"] = res
    return out
